# revision 13
# baseline (speedup 1.0000x reference)
"""log_matmul_exp(x, A) on 8 TRN2 NeuronCores — fp8 DoubleRow, int8 inputs.

out[n, e] = logsumexp_d(x[n, d] + A[d, e]) = log(exp(x) @ exp(A))

Sharding: 2 shards of N x 4 shards of E (N-heavy on purpose: the whole A
shard gates every PE batch, so the A shard is kept small — 1024x1024 — while
the x shard, which streams in batch-aligned slices, is 2048 wide). Per core:
xt [D=1024, ML=2048] and a [D=1024, EL=1024] arrive as int8 (v = q*S8,
|v| <= 5.8 covers N(0,1) tails); out [ML, EL] leaves bf16 (host -> fp32).

Compute scheme (validated on host, rel err ~2.3e-3 vs 2e-2 gate):
    ex8/ea8 = exp(q*S8 - 2.5) as fp8e4
        ACT path: scale+bias fused into ACTIVATE (TRN fp8e4 max normal is
        240, the shift keeps exp() in range).
        DVE path: exp bit-trick — z = q*k1 + k2 as int32, bitcast -> fp32
        ~= exp, copy -> fp8 — so both engines produce operands in parallel.
    s = ex8.T @ ea8   (PE, DoubleRow fp8: K=256/instruction, 216ns per
        512-row matmul = 155 TF/s -> 27.6us/core; the compute roofline)
    out = ln(s) + 5.0 via the ln bit-trick on the fp32 BITS of PSUM
        (bits*ln2/2^23 + c). On DVE: one tensor_scalar. On ACT: ACTIVATE
        Identity with the same scale/bias — Identity is in every activation
        table set, so ACT can interleave exps and lns with NO table reloads.
        Each batch's epilogue is split DVE/ACT (first/second mt pair).

PE choreography (from traces): ~7us fixed engine preamble; SP issues DMAs
serially at ~0.65us each (GpSimd SWDGE DMAs measured ~3us/transfer — do NOT
use); DMA issue->data ~4us. All batches kc-outer/t-inner (consecutive
matmuls on different PSUM banks sustain 216ns; same-bank back-to-back
measured 259ns and de-ramps the PE clock gate). Batch 0 consumes k-chunks
in arrival order (A0, A1 on ACT; A3 on DVE; A2 last). Warm-up matmuls on
DMA'd weights ramp the clock without delaying the first real matmul.
"""

import os
import sys

import numpy as np

for _p in ("/opt/trn_rl_repo", "/root/.axon_site/_ro/trn_rl_repo"):
    if os.path.isdir(_p) and _p not in sys.path:
        sys.path.insert(0, _p)

P = 128
D = 1024
N_FULL = 4096
E_FULL = 4096
GRID_N = 2
GRID_E = 4
N_CORES = GRID_N * GRID_E
ML = N_FULL // GRID_N  # 2048 local output rows
EL = E_FULL // GRID_E  # 1024 local output cols
KC = D // (2 * P)  # 4 contraction chunks of 256 (paired for DoubleRow)
NT = 512  # matmul moving free dim (one PSUM bank of fp32)
MT = ML // P  # 16 row tiles
ET = EL // NT  # 2 col tiles
GW = 4 * P  # x columns per streamed group (= one 4-row-tile batch)

S8 = 5.8 / 127.0  # int8 quantization step for x/A
SHIFT = 2.5  # exp(v - SHIFT); final out = ln(s) + 2*SHIFT
LN2 = 0.6931471805599453
EPS = 0.0573  # mean of log2(1+t)-t, centers the bit-trick approximations
LN_S1 = LN2 / (1 << 23)
LN_S2 = 2.0 * SHIFT - (127.0 - EPS) * LN2
EXP_K1 = S8 / LN2 * (1 << 23)
EXP_K2 = (127.0 - EPS) * (1 << 23) - SHIFT / LN2 * (1 << 23)

WARMUPS = 10
KC_ORDER = (0, 1, 3, 2)  # batch-0 k-chunk consumption in arrival order

_cache: dict = {}


def _build():
    import concourse.tile as tile
    from concourse import bacc, mybir

    AF = mybir.ActivationFunctionType
    DR = mybir.MatmulPerfMode.DoubleRow
    ALU = mybir.AluOpType
    f32 = mybir.dt.float32
    bf16 = mybir.dt.bfloat16
    f8 = mybir.dt.float8e4
    i32 = mybir.dt.int32
    i8 = mybir.dt.int8

    nc = bacc.Bacc(
        "TRN2",
        target_bir_lowering=False,
        debug=False,
        num_devices=N_CORES,
        num_swdge_queues=4,
        dynamic_dma_scratch_size=256,
    )
    xt = nc.dram_tensor("xt", [D, ML], i8, kind="ExternalInput")
    a = nc.dram_tensor("a", [D, EL], i8, kind="ExternalInput")
    wrm = nc.dram_tensor("wrm", [P, 2 * NT], f8, kind="ExternalInput")
    out = nc.dram_tensor("out", [ML, EL], bf16, kind="ExternalOutput")

    # d = kc*256 + sub*128 + p: paired-k layout for DoubleRow matmuls.
    xt3 = xt[:].rearrange("(kc sub p) m -> p kc sub m", p=P, sub=2)
    a3 = a[:].rearrange("(kc sub p) e -> p kc sub e", p=P, sub=2)
    wrm2 = wrm[:].rearrange("p (sub n) -> p sub n", sub=2)
    # output rows grouped in mt pairs: one [128, 2, EL] SBUF tile per pair
    out4 = out[:].rearrange("(q s p) e -> p q s e", p=P, s=2)

    with tile.TileContext(nc) as tc:
        with (
            tc.tile_pool(name="persist", bufs=1) as persist,
            tc.tile_pool(name="outp", bufs=4) as outp,
            tc.tile_pool(name="psum", bufs=8, space="PSUM") as psum_pool,
            tc.tile_pool(name="stage", bufs=4) as stage,
        ):
            wm = persist.tile([P, 2, NT], f8, tag="warm")
            nbias = persist.tile([P, 1], f32, tag="nbias")
            lbias = persist.tile([P, 1], f32, tag="lbias")
            scr = persist.tile([P, 1], f32, tag="scr")

            stx = [
                stage.tile([P, 2, ML], i8, tag="stx", name=f"stx{k}")
                for k in range(KC)
            ]
            sta = [
                stage.tile([P, 2, EL], i8, tag="sta", name=f"sta{k}")
                for k in range(KC)
            ]
            ex8 = [
                persist.tile([P, 2, ML], f8, tag=f"ex{k}", name=f"ex8_{k}")
                for k in range(KC)
            ]
            ea8 = [
                persist.tile([P, 2, EL], f8, tag=f"ea{k}", name=f"ea8_{k}")
                for k in range(KC)
            ]
            zint = persist.tile([P, 2, EL], i32, tag="zint")

            pss_b0 = [
                psum_pool.tile([P, NT], f32, tag="ps", name=f"ps_0_{t}")
                for t in range(8)
            ]

            # --- DMA issue (all on SP) -----------------------------------
            nc.gpsimd.memset(nbias[:], -SHIFT)
            nc.gpsimd.memset(lbias[:], LN_S2)
            nc.sync.dma_start(wm[:], wrm2)
            nc.sync.dma_start(stx[0][:, :, 0:GW], xt3[:, 0, :, 0:GW])
            nc.sync.dma_start(sta[0][:], a3[:, 0])
            nc.sync.dma_start(stx[1][:, :, 0:GW], xt3[:, 1, :, 0:GW])
            nc.sync.dma_start(sta[1][:], a3[:, 1])
            nc.sync.dma_start(stx[2][:, :, 0:GW], xt3[:, 2, :, 0:GW])
            nc.sync.dma_start(sta[3][:], a3[:, 3])
            nc.sync.dma_start(stx[3][:, :, 0:GW], xt3[:, 3, :, 0:GW])
            nc.sync.dma_start(sta[2][:], a3[:, 2])
            for g in range(1, MT // 4):
                sl = slice(g * GW, (g + 1) * GW)
                for kc in range(KC):
                    nc.sync.dma_start(stx[kc][:, :, sl], xt3[:, kc, :, sl])

            # --- PE warm-up into batch 0's first PSUM bank ---------------
            for _ in range(WARMUPS):
                nc.tensor.matmul(
                    pss_b0[0][:],
                    lhsT=wm[:, :, :P],
                    rhs=wm[:],
                    start=True,
                    stop=True,
                    perf_mode=DR,
                )

            # --- DVE stream: xg0, A3, xg1-kc23, then per-batch lns -------
            def dve_exp(dst, src, zsl):
                nc.vector.tensor_scalar(
                    out=zsl,
                    in0=src,
                    scalar1=EXP_K1,
                    scalar2=EXP_K2,
                    op0=ALU.mult,
                    op1=ALU.add,
                )
                nc.vector.tensor_copy(dst, zsl.bitcast(f32))

            def act_exp(dst, src):
                nc.scalar.activation(dst, src, AF.Exp, bias=nbias[:], scale=S8)

            for kc in range(KC):
                dve_exp(
                    ex8[kc][:, :, 0:GW], stx[kc][:, :, 0:GW], zint[:, :, 0:GW]
                )
            dve_exp(ea8[3][:], sta[3][:], zint[:])
            for kc in (2, 3):
                dve_exp(
                    ex8[kc][:, :, GW : 2 * GW],
                    stx[kc][:, :, GW : 2 * GW],
                    zint[:, :, 0:GW],
                )

            # --- ACT stream: table hoist, A0 (nt-sliced), A1, A2,
            # xg1-kc01; x groups 2-3 and lns interleave in the batch loop --
            nc.scalar.activation(scr[:], nbias[:], AF.Exp)
            for q in range(0, EL, NT):
                act_exp(ea8[0][:, :, q : q + NT], sta[0][:, :, q : q + NT])
            act_exp(ea8[1][:], sta[1][:])
            act_exp(ea8[2][:], sta[2][:])
            for kc in (0, 1):
                act_exp(ex8[kc][:, :, GW : 2 * GW], stx[kc][:, :, GW : 2 * GW])

            # --- matmul batches + split epilogue -------------------------
            # Batch = 4 row tiles x 2 col tiles = 8 PSUM banks, k-depth 4
            # accumulated in place, kc-outer/t-inner (216ns PE stream).
            # Epilogue: first mt pair -> DVE bit-ln; second -> ACT bit-ln
            # (Identity, tableless). One output DMA per mt pair. The x
            # group for batch b+2 is exp'd between batch b's mms and lns.
            for b in range(MT // 4):
                mts = tuple(4 * b + i for i in range(4))
                pss = (
                    pss_b0
                    if b == 0
                    else [
                        psum_pool.tile([P, NT], f32, tag="ps", name=f"ps_{b}_{t}")
                        for t in range(8)
                    ]
                )
                obp = [
                    outp.tile([P, 2, EL], bf16, tag="ob", name=f"ob_{b}_{i}")
                    for i in range(2)
                ]
                for kc in KC_ORDER:
                    for t in range(8):
                        nc.tensor.matmul(
                            pss[t][:],
                            lhsT=ex8[kc][
                                :, :, mts[t // 2] * P : (mts[t // 2] + 1) * P
                            ],
                            rhs=ea8[kc][:, :, (t % 2) * NT : (t % 2 + 1) * NT],
                            start=(kc == KC_ORDER[0]),
                            stop=(kc == KC_ORDER[-1]),
                            perf_mode=DR,
                        )
                # keep the x-group pipeline ahead: exp group b+2 now
                g = b + 2
                if g < MT // 4:
                    sl = slice(g * GW, (g + 1) * GW)
                    for kc in (0, 1):
                        act_exp(ex8[kc][:, :, sl], stx[kc][:, :, sl])
                    for kc in (2, 3):
                        dve_exp(
                            ex8[kc][:, :, sl], stx[kc][:, :, sl], zint[:, :, 0:GW]
                        )
                for t in range(4):  # first mt pair on DVE
                    nc.vector.tensor_scalar(
                        out=obp[0][:, t // 2, (t % 2) * NT : (t % 2 + 1) * NT],
                        in0=pss[t][:].bitcast(i32),
                        scalar1=LN_S1,
                        scalar2=LN_S2,
                        op0=ALU.mult,
                        op1=ALU.add,
                    )
                nc.sync.dma_start(out4[:, 2 * b], obp[0][:])
                for t in range(4, 8):  # second mt pair on ACT (tableless)
                    nc.scalar.activation(
                        obp[1][:, (t - 4) // 2, (t % 2) * NT : (t % 2 + 1) * NT],
                        pss[t][:].bitcast(i32),
                        AF.Identity,
                        bias=lbias[:],
                        scale=LN_S1,
                    )
                nc.sync.dma_start(out4[:, 2 * b + 1], obp[1][:])
    nc.compile()
    return nc


def _shard_inputs(x: np.ndarray, A: np.ndarray) -> list[dict]:
    import ml_dtypes

    xq = np.clip(np.rint(np.asarray(x) / S8), -127, 127).astype(np.int8)
    Aq = np.clip(np.rint(np.asarray(A) / S8), -127, 127).astype(np.int8)
    xT = np.ascontiguousarray(xq.T)  # (D, N)
    ones = np.ones((P, 2 * NT), dtype=ml_dtypes.float8_e4m3)
    in_maps = []
    for c in range(N_CORES):
        i, j = divmod(c, GRID_E)
        in_maps.append(
            {
                "xt": np.ascontiguousarray(xT[:, i * ML : (i + 1) * ML]),
                "a": np.ascontiguousarray(Aq[:, j * EL : (j + 1) * EL]),
                "wrm": ones,
            }
        )
    return in_maps


def _run(x: np.ndarray, A: np.ndarray, trace: bool = False):
    from concourse import bass_utils

    nc = _cache.get("nc")
    if nc is None:
        nc = _build()
        _cache["nc"] = nc

    in_maps = _shard_inputs(np.asarray(x), np.asarray(A))
    res = bass_utils.run_bass_kernel_spmd(
        nc, in_maps, list(range(N_CORES)), trace=trace
    )
    out = np.empty((N_FULL, E_FULL), dtype=np.float32)
    for c in range(N_CORES):
        i, j = divmod(c, GRID_E)
        out[i * ML : (i + 1) * ML, j * EL : (j + 1) * EL] = np.asarray(
            res.results[c]["out"]
        ).astype(np.float32)
    return out, res


def kernel(x: np.ndarray, A: np.ndarray) -> np.ndarray:
    out, _ = _run(x, A, trace=False)
    return out


# revision 14
# speedup vs baseline: 1.0734x; 1.0734x over previous
"""log_matmul_exp(x, A) on 8 TRN2 NeuronCores — fp8 DoubleRow, int8 inputs.

out[n, e] = logsumexp_d(x[n, d] + A[d, e]) = log(exp(x) @ exp(A))

Sharding: 2 shards of N x 4 shards of E (N-heavy on purpose: the whole A
shard gates every PE batch, so the A shard is kept small — 1024x1024 — while
the x shard, which streams in batch-aligned slices, is 2048 wide). Per core:
xt [D=1024, ML=2048] and a [D=1024, EL=1024] arrive as int8 (v = q*S8,
|v| <= 5.8 covers N(0,1) tails); out [ML, EL] leaves bf16 (host -> fp32).

Compute scheme (validated on host, rel err ~2.3e-3 vs 2e-2 gate):
    ex8/ea8 = exp(q*S8 - 2.5) as fp8e4
        ACT path: scale+bias fused into ACTIVATE (TRN fp8e4 max normal is
        240, the shift keeps exp() in range).
        DVE path: exp bit-trick — z = q*k1 + k2 as int32, bitcast -> fp32
        ~= exp, copy -> fp8 — so both engines produce operands in parallel.
    s = ex8.T @ ea8   (PE, DoubleRow fp8: K=256/instruction, 216ns per
        512-row matmul = 155 TF/s -> 27.6us/core; the compute roofline)
    out = ln(s) + 5.0 via the ln bit-trick on the fp32 BITS of PSUM
        (bits*ln2/2^23 + c). On DVE: one tensor_scalar. On ACT: ACTIVATE
        Identity with the same scale/bias — Identity is in every activation
        table set, so ACT can interleave exps and lns with NO table reloads.
        Each batch's epilogue is split DVE/ACT (first/second mt pair).

PE choreography (from traces): ~7us fixed engine preamble; SP issues DMAs
serially at ~0.65us each (GpSimd SWDGE DMAs measured ~3us/transfer — do NOT
use); DMA issue->data ~4us. All batches kc-outer/t-inner (consecutive
matmuls on different PSUM banks sustain 216ns; same-bank back-to-back
measured 259ns and de-ramps the PE clock gate). Batch 0 consumes k-chunks
in arrival order (A0, A1 on ACT; A3 on DVE; A2 last). Warm-up matmuls on
DMA'd weights ramp the clock without delaying the first real matmul.
"""

import os
import sys

import numpy as np

for _p in ("/opt/trn_rl_repo", "/root/.axon_site/_ro/trn_rl_repo"):
    if os.path.isdir(_p) and _p not in sys.path:
        sys.path.insert(0, _p)

P = 128
D = 1024
N_FULL = 4096
E_FULL = 4096
GRID_N = 2
GRID_E = 4
N_CORES = GRID_N * GRID_E
ML = N_FULL // GRID_N  # 2048 local output rows
EL = E_FULL // GRID_E  # 1024 local output cols
KC = D // (2 * P)  # 4 contraction chunks of 256 (paired for DoubleRow)
NT = 512  # matmul moving free dim (one PSUM bank of fp32)
MT = ML // P  # 16 row tiles
ET = EL // NT  # 2 col tiles
GW = 4 * P  # x columns per streamed group (= one 4-row-tile batch)

S8 = 5.8 / 127.0  # int8 quantization step for x/A
SHIFT = 2.5  # exp(v - SHIFT); final out = ln(s) + 2*SHIFT
LN2 = 0.6931471805599453
EPS = 0.0573  # mean of log2(1+t)-t, centers the bit-trick approximations
LN_S1 = LN2 / (1 << 23)
LN_S2 = 2.0 * SHIFT - (127.0 - EPS) * LN2
EXP_K1 = S8 / LN2 * (1 << 23)
EXP_K2 = (127.0 - EPS) * (1 << 23) - SHIFT / LN2 * (1 << 23)

WARMUPS = 10
KC_ORDER = (0, 1, 3, 2)  # batch-0 k-chunk consumption in arrival order

_cache: dict = {}


def _build():
    import concourse.tile as tile
    from concourse import bacc, mybir

    AF = mybir.ActivationFunctionType
    DR = mybir.MatmulPerfMode.DoubleRow
    ALU = mybir.AluOpType
    f32 = mybir.dt.float32
    bf16 = mybir.dt.bfloat16
    f8 = mybir.dt.float8e4
    i32 = mybir.dt.int32
    i8 = mybir.dt.int8

    nc = bacc.Bacc(
        "TRN2",
        target_bir_lowering=False,
        debug=False,
        num_devices=N_CORES,
        num_swdge_queues=4,
        dynamic_dma_scratch_size=256,
    )
    xt = nc.dram_tensor("xt", [D, ML], i8, kind="ExternalInput")
    a = nc.dram_tensor("a", [D, EL], i8, kind="ExternalInput")
    wrm = nc.dram_tensor("wrm", [P, 2 * NT], f8, kind="ExternalInput")
    out = nc.dram_tensor("out", [ML, EL], bf16, kind="ExternalOutput")

    # d = kc*256 + sub*128 + p: paired-k layout for DoubleRow matmuls.
    xt3 = xt[:].rearrange("(kc sub p) m -> p kc sub m", p=P, sub=2)
    a3 = a[:].rearrange("(kc sub p) e -> p kc sub e", p=P, sub=2)
    wrm2 = wrm[:].rearrange("p (sub n) -> p sub n", sub=2)
    # output rows grouped in mt pairs: one [128, 2, EL] SBUF tile per pair
    out4 = out[:].rearrange("(q s p) e -> p q s e", p=P, s=2)

    with tile.TileContext(nc) as tc:
        with (
            tc.tile_pool(name="persist", bufs=1) as persist,
            tc.tile_pool(name="outp", bufs=4) as outp,
            tc.tile_pool(name="psum", bufs=8, space="PSUM") as psum_pool,
            tc.tile_pool(name="stage", bufs=4) as stage,
        ):
            wm = persist.tile([P, 2, NT], f8, tag="warm")
            nbias = persist.tile([P, 1], f32, tag="nbias")
            lbias = persist.tile([P, 1], f32, tag="lbias")
            scr = persist.tile([P, 1], f32, tag="scr")

            stx = [
                stage.tile([P, 2, ML], i8, tag="stx", name=f"stx{k}")
                for k in range(KC)
            ]
            sta = [
                stage.tile([P, 2, EL], i8, tag="sta", name=f"sta{k}")
                for k in range(KC)
            ]
            ex8 = [
                persist.tile([P, 2, ML], f8, tag=f"ex{k}", name=f"ex8_{k}")
                for k in range(KC)
            ]
            ea8 = [
                persist.tile([P, 2, EL], f8, tag=f"ea{k}", name=f"ea8_{k}")
                for k in range(KC)
            ]
            zint = persist.tile([P, 2, EL], i32, tag="zint")
            zint2 = persist.tile([P, 2, GW], i32, tag="zint2")

            pss_b0 = [
                psum_pool.tile([P, NT], f32, tag="ps", name=f"ps_0_{t}")
                for t in range(8)
            ]

            # --- DMA issue (all on SP) -----------------------------------
            nc.gpsimd.memset(nbias[:], -SHIFT)
            nc.gpsimd.memset(lbias[:], LN_S2)
            nc.sync.dma_start(wm[:], wrm2)
            nc.sync.dma_start(stx[0][:, :, 0:GW], xt3[:, 0, :, 0:GW])
            nc.sync.dma_start(sta[0][:], a3[:, 0])
            nc.sync.dma_start(stx[1][:, :, 0:GW], xt3[:, 1, :, 0:GW])
            nc.sync.dma_start(sta[1][:], a3[:, 1])
            nc.sync.dma_start(stx[2][:, :, 0:GW], xt3[:, 2, :, 0:GW])
            nc.sync.dma_start(sta[3][:], a3[:, 3])
            nc.sync.dma_start(stx[3][:, :, 0:GW], xt3[:, 3, :, 0:GW])
            nc.sync.dma_start(sta[2][:], a3[:, 2])
            for g in range(1, MT // 4):
                sl = slice(g * GW, (g + 1) * GW)
                for kc in range(KC):
                    nc.sync.dma_start(stx[kc][:, :, sl], xt3[:, kc, :, sl])

            # --- PE warm-up into batch 0's first PSUM bank ---------------
            for _ in range(WARMUPS):
                nc.tensor.matmul(
                    pss_b0[0][:],
                    lhsT=wm[:, :, :P],
                    rhs=wm[:],
                    start=True,
                    stop=True,
                    perf_mode=DR,
                )

            # --- DVE stream: xg0, A3, xg1-kc23, then per-batch lns -------
            def dve_exp(dst, src, zsl):
                nc.vector.tensor_scalar(
                    out=zsl,
                    in0=src,
                    scalar1=EXP_K1,
                    scalar2=EXP_K2,
                    op0=ALU.mult,
                    op1=ALU.add,
                )
                nc.vector.tensor_copy(dst, zsl.bitcast(f32))

            def act_exp(dst, src):
                nc.scalar.activation(dst, src, AF.Exp, bias=nbias[:], scale=S8)

            for kc in range(KC):
                dve_exp(
                    ex8[kc][:, :, 0:GW], stx[kc][:, :, 0:GW], zint[:, :, 0:GW]
                )
            dve_exp(ea8[3][:], sta[3][:], zint[:])
            for kc in (0, 1):
                dve_exp(
                    ex8[kc][:, :, GW : 2 * GW],
                    stx[kc][:, :, GW : 2 * GW],
                    zint[:, :, 0:GW],
                )

            # GPS: x group 3 via the same exp bit-trick (Pool never
            # throttles the PE clock; DVE busy >~60% measurably does)
            for kc in range(KC):
                sl3 = slice(3 * GW, 4 * GW)
                nc.gpsimd.tensor_scalar(
                    out=zint2[:],
                    in0=stx[kc][:, :, sl3],
                    scalar1=EXP_K1,
                    scalar2=EXP_K2,
                    op0=ALU.mult,
                    op1=ALU.add,
                )
                nc.gpsimd.tensor_copy(ex8[kc][:, :, sl3], zint2[:].bitcast(f32))

            # --- ACT stream: table hoist, A0 (nt-sliced), A1, A2,
            # xg1-kc01; x groups 2-3 and lns interleave in the batch loop --
            nc.scalar.activation(scr[:], nbias[:], AF.Exp)
            for q in range(0, EL, NT):
                act_exp(ea8[0][:, :, q : q + NT], sta[0][:, :, q : q + NT])
            act_exp(ea8[1][:], sta[1][:])
            act_exp(ea8[2][:], sta[2][:])
            for kc in (2, 3):
                act_exp(ex8[kc][:, :, GW : 2 * GW], stx[kc][:, :, GW : 2 * GW])

            # --- matmul batches + split epilogue -------------------------
            # Batch = 4 row tiles x 2 col tiles = 8 PSUM banks, k-depth 4
            # accumulated in place, kc-outer/t-inner (216ns PE stream).
            # Epilogue: first mt pair -> DVE bit-ln; second -> ACT bit-ln
            # (Identity, tableless). One output DMA per mt pair. The x
            # group for batch b+2 is exp'd between batch b's mms and lns.
            for b in range(MT // 4):
                mts = tuple(4 * b + i for i in range(4))
                pss = (
                    pss_b0
                    if b == 0
                    else [
                        psum_pool.tile([P, NT], f32, tag="ps", name=f"ps_{b}_{t}")
                        for t in range(8)
                    ]
                )
                obp = [
                    outp.tile([P, 2, EL], bf16, tag="ob", name=f"ob_{b}_{i}")
                    for i in range(2)
                ]
                for kc in KC_ORDER:
                    for t in range(8):
                        nc.tensor.matmul(
                            pss[t][:],
                            lhsT=ex8[kc][
                                :, :, mts[t // 2] * P : (mts[t // 2] + 1) * P
                            ],
                            rhs=ea8[kc][:, :, (t % 2) * NT : (t % 2 + 1) * NT],
                            start=(kc == KC_ORDER[0]),
                            stop=(kc == KC_ORDER[-1]),
                            perf_mode=DR,
                        )
                for t in range(4):  # first mt pair on DVE
                    nc.vector.tensor_scalar(
                        out=obp[0][:, t // 2, (t % 2) * NT : (t % 2 + 1) * NT],
                        in0=pss[t][:].bitcast(i32),
                        scalar1=LN_S1,
                        scalar2=LN_S2,
                        op0=ALU.mult,
                        op1=ALU.add,
                    )
                nc.sync.dma_start(out4[:, 2 * b], obp[0][:])
                for t in range(4, 8):  # second mt pair on ACT (tableless)
                    nc.scalar.activation(
                        obp[1][:, (t - 4) // 2, (t % 2) * NT : (t % 2 + 1) * NT],
                        pss[t][:].bitcast(i32),
                        AF.Identity,
                        bias=lbias[:],
                        scale=LN_S1,
                    )
                nc.sync.dma_start(out4[:, 2 * b + 1], obp[1][:])
                if b == 0:  # x group 2 on ACT once batch 0's lns are out
                    sl = slice(2 * GW, 3 * GW)
                    for kc in range(KC):
                        act_exp(ex8[kc][:, :, sl], stx[kc][:, :, sl])
    nc.compile()
    return nc


def _shard_inputs(x: np.ndarray, A: np.ndarray) -> list[dict]:
    import ml_dtypes

    xq = np.clip(np.rint(np.asarray(x) / S8), -127, 127).astype(np.int8)
    Aq = np.clip(np.rint(np.asarray(A) / S8), -127, 127).astype(np.int8)
    xT = np.ascontiguousarray(xq.T)  # (D, N)
    ones = np.ones((P, 2 * NT), dtype=ml_dtypes.float8_e4m3)
    in_maps = []
    for c in range(N_CORES):
        i, j = divmod(c, GRID_E)
        in_maps.append(
            {
                "xt": np.ascontiguousarray(xT[:, i * ML : (i + 1) * ML]),
                "a": np.ascontiguousarray(Aq[:, j * EL : (j + 1) * EL]),
                "wrm": ones,
            }
        )
    return in_maps


def _run(x: np.ndarray, A: np.ndarray, trace: bool = False):
    from concourse import bass_utils

    nc = _cache.get("nc")
    if nc is None:
        nc = _build()
        _cache["nc"] = nc

    in_maps = _shard_inputs(np.asarray(x), np.asarray(A))
    res = bass_utils.run_bass_kernel_spmd(
        nc, in_maps, list(range(N_CORES)), trace=trace
    )
    out = np.empty((N_FULL, E_FULL), dtype=np.float32)
    for c in range(N_CORES):
        i, j = divmod(c, GRID_E)
        out[i * ML : (i + 1) * ML, j * EL : (j + 1) * EL] = np.asarray(
            res.results[c]["out"]
        ).astype(np.float32)
    return out, res


def kernel(x: np.ndarray, A: np.ndarray) -> np.ndarray:
    out, _ = _run(x, A, trace=False)
    return out
